# revision 27
# baseline (speedup 1.0000x reference)
"""Trainium2 Bass kernel: single-layer causal attention block (q/k/v/o + RoPE).

v2 design — minimize host work + host<->device traffic, bf16 device compute:
  Sharding: 8 cores = 2 batches x 4 head-groups (4 heads each).
  Host uploads per core are CONTIGUOUS VIEWS of the original arrays only
  (plus tiny RoPE tables): the S/4 slice of x[b], half of the head-group's
  q/k/v weight rows (pair core has the other half), 128 rows of o_proj.
  On device:
    - AllGather(4) assembles full x[b]; AllGather(2) the q/k/v weight
      slices; AllToAll(8) deals each core its o_proj column block.
    - PE-transposes (+ f32->bf16 cast on PSUM copy-out) build x^T and
      W^T in SBUF.
    - RoPE in interleaved layout via a pair-swap permutation matmul and
      sign-folded cos/sin tables (no even/odd weight permutation).
    - scores^T = K^T-stationary bf16 matmuls (64-partition lhsT, one mm
      per key tile), exp on ACT with scale=1/8, multiplicative causal
      mask (built on-device via affine_select) on diagonal tiles,
      PV with a ones-column in V for the softmax row sums.
    - o_proj partials per 512-row chunk, ReduceScatter(add, 4) sums the
      head-group partials on device; each core outputs a [512, 1024] f32
      slice of the final output.
  Host gathers 8 x [512,1024] f32 pieces into [2, 2048, 1024].
"""

import os
import sys

import numpy as np

sys.path.insert(0, "/opt/trn_rl_repo")

import concourse.bass as bass  # noqa: E402
import concourse.tile as tile  # noqa: E402
from concourse import bacc, mybir  # noqa: E402
from concourse import bass_utils  # noqa: E402

B, S, D, H, DK = 2, 2048, 1024, 16, 64
NCORES = 8
HPC = H // 4     # 4 heads per core
CW = HPC * DK    # 256 head dims per core
VW = DK + 1      # 65: v width per head incl ones column
ND = D // 128    # 8 contraction chunks
NS = S // 128    # 16 s-tiles
NSC = S // 512   # 4 s-chunks
SQ = 512         # s-chunk size
ROPE_THETA = 10000.0

F32 = mybir.dt.float32
BF16 = mybir.dt.bfloat16
EXP = mybir.ActivationFunctionType.Exp

G4 = [[0, 1, 2, 3], [4, 5, 6, 7]]
G2 = [[0, 4], [1, 5], [2, 6], [3, 7]]
G8 = [[0, 1, 2, 3, 4, 5, 6, 7]]


def _build_kernel(tc, nc, t, dbg=False):
    from contextlib import ExitStack
    stack = ExitStack()

    csb, snb, psw = t["csb"], t["snb"], t["psw"]
    out = t["out"]
    xg, qkvg, woout = t["xg"], t["qkvg"], t["woout"]
    part, rsout = t["part"], t["rsout"]

    if not dbg:
        x_sh, wq_h, wk_h, wv_h, wo_sh = (t["x_sh"], t["wq_h"], t["wk_h"],
                                         t["wv_h"], t["wo_sh"])
        xcc, qkvcc, woin = t["xcc"], t["qkvcc"], t["woin"]
        # ---- Phase A: stage collective inputs (DRAM->DRAM) + launch ----
        nc.sync.dma_start(xcc[:], x_sh[:])
        nc.sync.dma_start(qkvcc[0:128, :], wq_h[:])
        nc.sync.dma_start(qkvcc[128:256, :], wk_h[:])
        nc.sync.dma_start(qkvcc[256:384, :], wv_h[:])
        # o_proj AllToAll input: shard j (rows j*128..) = wo_sh[:, (j%4)*256..]
        # so that core j receives o_proj[:, (j%4)*256:(j%4+1)*256] stacked
        # over all 8 contributors' row blocks.
        for j in range(8):
            nc.sync.dma_start(woin[j * 128:(j + 1) * 128, :],
                              wo_sh[:, (j % 4) * CW:(j % 4 + 1) * CW])
        nc.gpsimd.collective_compute(
            "AllGather", mybir.AluOpType.bypass, replica_groups=G4,
            ins=[xcc[:]], outs=[xg[:]])
        nc.gpsimd.collective_compute(
            "AllGather", mybir.AluOpType.bypass, replica_groups=G2,
            ins=[qkvcc[:]], outs=[qkvg[:]])
        nc.gpsimd.collective_compute(
            "AllToAll", mybir.AluOpType.bypass, replica_groups=G8,
            ins=[woin[:]], outs=[woout[:]])

    constp = stack.enter_context(tc.tile_pool(name="const", bufs=1))
    pers = stack.enter_context(tc.tile_pool(name="persist", bufs=1))

    # ---- constants: RoPE tables, pair-swap matrix, causal masks ----
    cs_sb = constp.tile([128, S], BF16)
    sn_sb = constp.tile([128, S], BF16)
    nc.sync.dma_start(cs_sb[0:64, :], csb[:])
    nc.sync.dma_start(cs_sb[64:128, :], csb[:])
    nc.sync.dma_start(sn_sb[0:64, :], snb[:])
    nc.sync.dma_start(sn_sb[64:128, :], snb[:])
    psw_sb = constp.tile([128, 128], BF16)
    nc.sync.dma_start(psw_sb[:], psw[:])
    one_sb = constp.tile([128, SQ], BF16)
    nc.gpsimd.memset(one_sb[:], 1.0)
    mk_sb = constp.tile([128, NSC * SQ], BF16)
    for j in range(NSC):
        # keep 1.0 where free_idx - partition - 128*j >= 0, else 0
        nc.gpsimd.affine_select(
            mk_sb[:, j * SQ:(j + 1) * SQ], one_sb[:],
            pattern=[[1, SQ]], compare_op=mybir.AluOpType.is_ge,
            fill=0.0, base=-(128 * j), channel_multiplier=-1)

    # persistent tensors
    qrot = [pers.tile([128, S], BF16, name=f"qrot{i}") for i in range(2)]
    krot = [pers.tile([128, S], BF16, name=f"krot{i}") for i in range(2)]
    v_sb = pers.tile([128, NS * HPC * VW], BF16)
    ctx_sb = pers.tile([128, 2 * S], BF16)
    wo_sb = pers.tile([128, 2 * D], BF16)
    nc.gpsimd.memset(v_sb[:], 1.0)  # pre-fill the per-head ones columns

    # ---- Phase B: load f32 tiles, PE-transpose, cast-copy to bf16 SBUF ----
    id_sb = constp.tile([128, 128], F32)
    nc.sync.dma_start(id_sb[:], t["iden"][:])
    xtp = stack.enter_context(tc.tile_pool(name="xt", bufs=1))
    xt_sb = xtp.tile([128, ND * S], BF16)
    wtp2 = stack.enter_context(tc.tile_pool(name="wt", bufs=1))
    wT = [wtp2.tile([128, ND * CW], BF16, name=f"wT{w}") for w in range(3)]
    with tc.tile_pool(name="xload", bufs=3) as xlp, \
         tc.tile_pool(name="tps", bufs=4, space="PSUM") as tpp:
        for st in range(NS):
            xa = xlp.tile([128, D], F32, name=f"xa{st}", tag="xa")
            eng = nc.sync if st % 2 == 0 else nc.scalar
            eng.dma_start(xa[:], xg[st * 128:(st + 1) * 128, :])
            for d in range(ND):
                pt = tpp.tile([128, 128], F32, name=f"xt{st}_{d}", tag="tp")
                nc.tensor.transpose(pt[:], xa[:, d * 128:(d + 1) * 128],
                                    id_sb[:])
                dst = xt_sb[:, d * S + st * 128: d * S + (st + 1) * 128]
                if d % 2 == 0:
                    nc.vector.tensor_copy(dst, pt[:])
                else:
                    nc.scalar.copy(dst, pt[:])

        # ---- weights after AG2: same PE-transpose path ----
        for blk in range(6):
            w, half = blk % 3, blk // 3
            src_r = half * 384 + w * 128
            wa = xlp.tile([128, D], F32, name=f"wa{blk}", tag="xa")
            eng = nc.sync if blk % 2 == 0 else nc.scalar
            eng.dma_start(wa[:], qkvg[src_r:src_r + 128, :])
            for d in range(ND):
                pt = tpp.tile([128, 128], F32, name=f"wt{blk}_{d}", tag="tp")
                nc.tensor.transpose(pt[:], wa[:, d * 128:(d + 1) * 128],
                                    id_sb[:])
                dst = wT[w][:, d * CW + half * 128: d * CW + half * 128 + 128]
                if d % 2 == 0:
                    nc.vector.tensor_copy(dst, pt[:])
                else:
                    nc.scalar.copy(dst, pt[:])

        # ---- o_proj weight after AllToAll ----
        for jt in range(ND):
            oa = xlp.tile([128, CW], F32, name=f"oa{jt}", tag="xa")
            eng = nc.sync if jt % 2 == 0 else nc.scalar
            eng.dma_start(oa[:], woout[jt * 128:(jt + 1) * 128, :])
            for cb in range(2):
                pt = tpp.tile([128, 128], F32, name=f"ot{jt}_{cb}", tag="tp")
                nc.tensor.transpose(pt[:], oa[:, cb * 128:(cb + 1) * 128],
                                    id_sb[:])
                dst = wo_sb[:, cb * D + jt * 128: cb * D + (jt + 1) * 128]
                if jt % 2 == 0:
                    nc.vector.tensor_copy(dst, pt[:])
                else:
                    nc.scalar.copy(dst, pt[:])

    # ---- Phase C: projections (bf16 matmuls, f32 psum) + RoPE ----
    with tc.tile_pool(name="qraw", bufs=1) as qrp, \
         tc.tile_pool(name="pjps", bufs=4, space="PSUM") as pjps, \
         tc.tile_pool(name="swps", bufs=2, space="PSUM") as swps, \
         tc.tile_pool(name="vps", bufs=2, space="PSUM") as vps, \
         tc.tile_pool(name="ropet", bufs=4) as rtp:
        blocks = [(qrot[0], wT[0], 0), (qrot[1], wT[0], 128),
                  (krot[0], wT[1], 0), (krot[1], wT[1], 128)]
        qraw = [qrp.tile([128, S], BF16, name=f"qraw{i}") for i in range(4)]
        for bi, (dst, wsb, co) in enumerate(blocks):
            raw = qraw[bi]
            for sc in range(NSC):
                ps = pjps.tile([128, SQ], F32, name=f"pj{bi}_{sc}", tag="pj")
                for d in range(ND):
                    nc.tensor.matmul(
                        ps[:],
                        wsb[:, d * CW + co: d * CW + co + 128],
                        xt_sb[:, d * S + sc * SQ: d * S + (sc + 1) * SQ],
                        start=(d == 0), stop=(d == ND - 1))
                nc.scalar.copy(raw[:, sc * SQ:(sc + 1) * SQ], ps[:])
                # pair-swap via permutation matmul, then sign-folded rope
                psx = swps.tile([128, SQ], F32, name=f"sw{bi}_{sc}", tag="sw")
                nc.tensor.matmul(psx[:], psw_sb[:],
                                 raw[:, sc * SQ:(sc + 1) * SQ],
                                 start=True, stop=True)
                t1 = rtp.tile([128, SQ], BF16, name=f"t1_{bi}_{sc}", tag="t1")
                t2 = rtp.tile([128, SQ], BF16, name=f"t2_{bi}_{sc}", tag="t2")
                nc.vector.tensor_mul(t1[:], raw[:, sc * SQ:(sc + 1) * SQ],
                                     cs_sb[:, sc * SQ:(sc + 1) * SQ])
                nc.vector.tensor_mul(t2[:], psx[:],
                                     sn_sb[:, sc * SQ:(sc + 1) * SQ])
                nc.vector.tensor_add(dst[:, sc * SQ:(sc + 1) * SQ], t1[:], t2[:])
        for sm in range(NS):
            pv = vps.tile([128, CW], F32, name=f"pv{sm}", tag="pv")
            for d in range(ND):
                nc.tensor.matmul(
                    pv[:],
                    xt_sb[:, d * S + sm * 128: d * S + sm * 128 + 128],
                    wT[2][:, d * CW:(d + 1) * CW],
                    start=(d == 0), stop=(d == ND - 1))
            base = sm * HPC * VW
            dst3 = v_sb[:, base:base + HPC * VW].rearrange("p (h c) -> p h c", c=VW)
            nc.vector.tensor_copy(dst3[:, :, 0:DK],
                                  pv[:].rearrange("p (h c) -> p h c", c=DK))

    # ---- Phase D: attention (c-outer, h-inner) + o_proj + chunked RS ----
    with tc.tile_pool(name="sps", bufs=4, space="PSUM") as sps, \
         tc.tile_pool(name="cps", bufs=2, space="PSUM") as cps, \
         tc.tile_pool(name="ops", bufs=2, space="PSUM") as opsp, \
         tc.tile_pool(name="expool", bufs=6) as exp_pool, \
         tc.tile_pool(name="smp", bufs=8) as smp, \
         tc.tile_pool(name="obuf", bufs=4) as obp:
        for c in range(NSC):
            nsk = 4 * (c + 1)
            for h in range(HPC):
                T = h // 2
                r0 = (h % 2) * 64
                tp = (r0, 0)
                pctx = cps.tile([VW, SQ], F32, name=f"pc{c}_{h}", tag="pc")
                exps = []
                DEPTH = 3

                def pv_mm(ti, nsk=nsk, pctx=pctx, exps=exps, h=h):
                    vbase = ti * HPC * VW + h * VW
                    nc.tensor.matmul(
                        pctx[:],
                        v_sb[:, vbase:vbase + VW],
                        exps[ti][:],
                        start=(ti == 0), stop=(ti == nsk - 1),
                        skip_group_check=True)

                for ti in range(nsk):
                    pscore = sps.tile([128, SQ], F32, name=f"ps{c}_{h}_{ti}",
                                      tag="ps")
                    nc.tensor.matmul(
                        pscore[:],
                        krot[T][r0:r0 + 64, ti * 128:(ti + 1) * 128],
                        qrot[T][r0:r0 + 64, c * SQ:(c + 1) * SQ],
                        start=True, stop=True, tile_position=tp,
                        skip_group_check=True)
                    et = exp_pool.tile([128, SQ], BF16, name=f"e{c}_{h}_{ti}",
                                       tag="et")
                    nc.scalar.activation(et[:], pscore[:], EXP, scale=0.125)
                    j = ti - 4 * c
                    if j >= 0:
                        nc.vector.tensor_mul(et[:], et[:],
                                             mk_sb[:, j * SQ:(j + 1) * SQ])
                    exps.append(et)
                    if ti >= DEPTH:
                        pv_mm(ti - DEPTH)
                for ti in range(max(0, nsk - DEPTH), nsk):
                    pv_mm(ti)
                rc = smp.tile([1, SQ], F32, name=f"rc{c}_{h}", tag="rc")
                nc.vector.reciprocal(rc[:], pctx[DK:DK + 1, :])
                rb = smp.tile([64, SQ], F32, name=f"rb{c}_{h}", tag="rb")
                nc.gpsimd.partition_broadcast(rb[:], rc[:])
                nc.vector.tensor_mul(
                    ctx_sb[r0:r0 + 64,
                           (h // 2) * S + c * SQ:(h // 2) * S + (c + 1) * SQ],
                    pctx[0:DK, :], rb[:])
            # o_proj for this 512-row chunk, then ReduceScatter it
            for sm in range(4 * c, 4 * c + 4):
                pos = [opsp.tile([128, SQ], F32, name=f"op{sm}_{i}", tag="op")
                       for i in range(2)]
                for cb in range(2):
                    lhsT = ctx_sb[:, cb * S + sm * 128: cb * S + sm * 128 + 128]
                    for do_ in range(2):
                        nc.tensor.matmul(
                            pos[do_][:],
                            lhsT,
                            wo_sb[:, cb * D + do_ * SQ: cb * D + (do_ + 1) * SQ],
                            start=(cb == 0), stop=(cb == 1))
                ot = obp.tile([128, D], BF16, name=f"ot{sm}", tag="ot")
                nc.vector.tensor_copy(ot[:, 0:SQ], pos[0][:])
                nc.vector.tensor_copy(ot[:, SQ:D], pos[1][:])
                nc.sync.dma_start(part[sm * 128:(sm + 1) * 128, :], ot[:])
            if not dbg:
                nc.gpsimd.collective_compute(
                    "ReduceScatter", mybir.AluOpType.add, replica_groups=G4,
                    ins=[part[c * SQ:(c + 1) * SQ, :]],
                    outs=[rsout[c * 128:(c + 1) * 128, :]])

        if not dbg:
            # ---- output: upcast rsout chunks to f32 and store ----
            with tc.tile_pool(name="oq", bufs=4) as oqp:
                for j in range(NSC):
                    rt = oqp.tile([128, D], BF16, name=f"rt{j}", tag="rt")
                    nc.sync.dma_start(rt[:], rsout[j * 128:(j + 1) * 128, :])
                    rf = oqp.tile([128, D], F32, name=f"rf{j}", tag="rf")
                    nc.vector.tensor_copy(rf[:], rt[:])
                    nc.sync.dma_start(out[j * 128:(j + 1) * 128, :], rf[:])
        else:
            nc.sync.dma_start(t["dbg_xt"][:], xt_sb[:])
            nc.sync.dma_start(t["dbg_q"][:], qrot[0][:])
            nc.sync.dma_start(t["dbg_k"][:], krot[0][:])
            nc.sync.dma_start(t["dbg_v"][:], v_sb[:])
            nc.sync.dma_start(t["dbg_ctx"][:], ctx_sb[:])
            nc.sync.dma_start(t["dbg_wo"][:], wo_sb[:])
    stack.close()


def build_nc(dbg=False):
    nc = bacc.Bacc("TRN2", target_bir_lowering=False, debug=False,
                   enable_asserts=False, num_devices=NCORES)
    t = {}
    if not dbg:
        t["x_sh"] = nc.dram_tensor("x_sh", [SQ, D], F32, kind="ExternalInput").ap()
        t["wq_h"] = nc.dram_tensor("wq_h", [128, D], F32, kind="ExternalInput").ap()
        t["wk_h"] = nc.dram_tensor("wk_h", [128, D], F32, kind="ExternalInput").ap()
        t["wv_h"] = nc.dram_tensor("wv_h", [128, D], F32, kind="ExternalInput").ap()
        t["wo_sh"] = nc.dram_tensor("wo_sh", [128, D], F32,
                                    kind="ExternalInput").ap()
    t["csb"] = nc.dram_tensor("csb", [64, S], BF16, kind="ExternalInput").ap()
    t["snb"] = nc.dram_tensor("snb", [64, S], BF16, kind="ExternalInput").ap()
    t["psw"] = nc.dram_tensor("psw", [128, 128], BF16, kind="ExternalInput").ap()
    t["iden"] = nc.dram_tensor("iden", [128, 128], F32, kind="ExternalInput").ap()
    t["out"] = nc.dram_tensor("out", [SQ, D], F32, kind="ExternalOutput").ap()

    ikind = "ExternalInput" if dbg else "Internal"
    t["xg"] = nc.dram_tensor("xg", [S, D], F32, kind=ikind).ap()
    t["qkvg"] = nc.dram_tensor("qkvg", [768, D], F32, kind=ikind).ap()
    t["woout"] = nc.dram_tensor("woout", [D, CW], F32, kind=ikind).ap()
    if not dbg:
        t["xcc"] = nc.dram_tensor("xcc", [SQ, D], F32, kind="Internal").ap()
        t["qkvcc"] = nc.dram_tensor("qkvcc", [384, D], F32, kind="Internal").ap()
        t["woin"] = nc.dram_tensor("woin", [D, CW], F32, kind="Internal").ap()
    t["part"] = nc.dram_tensor("part", [S, D], BF16,
                               kind="ExternalOutput" if dbg else "Internal").ap()
    t["rsout"] = nc.dram_tensor("rsout", [SQ, D], BF16, kind="Internal").ap()
    if dbg:
        t["dbg_xt"] = nc.dram_tensor("dbg_xt", [128, ND * S], BF16,
                                     kind="ExternalOutput").ap()
        t["dbg_q"] = nc.dram_tensor("dbg_q", [128, S], BF16,
                                    kind="ExternalOutput").ap()
        t["dbg_k"] = nc.dram_tensor("dbg_k", [128, S], BF16,
                                    kind="ExternalOutput").ap()
        t["dbg_v"] = nc.dram_tensor("dbg_v", [128, NS * HPC * VW], BF16,
                                    kind="ExternalOutput").ap()
        t["dbg_ctx"] = nc.dram_tensor("dbg_ctx", [128, 2 * S], BF16,
                                      kind="ExternalOutput").ap()
        t["dbg_wo"] = nc.dram_tensor("dbg_wo", [128, 2 * D], BF16,
                                     kind="ExternalOutput").ap()
    with tile.TileContext(nc) as tc:
        _build_kernel(tc, nc, t, dbg=dbg)
    nc.compile()
    return nc


_TABLES = []


def _tables(token_positions):
    if _TABLES:
        return _TABLES[0]
    import ml_dtypes
    pos = np.asarray(token_positions).astype(np.float64)
    inv = ROPE_THETA ** (-2.0 * np.arange(DK // 2, dtype=np.float64) / DK)
    ang = inv[:, None] * pos[None, :]                    # [32, S]
    cs = np.repeat(np.cos(ang), 2, axis=0)               # [64, S]
    sn = np.repeat(np.sin(ang), 2, axis=0)
    sn[0::2] *= -1.0                                     # sign-folded
    csb = cs.astype(ml_dtypes.bfloat16)
    snb = sn.astype(ml_dtypes.bfloat16)
    idx = np.arange(128)
    psw = (idx[:, None] == (idx[None, :] ^ 1)).astype(ml_dtypes.bfloat16)
    iden = np.eye(128, dtype=np.float32)
    _TABLES.append((csb, snb, psw, iden))
    return _TABLES[0]


def make_in_maps(in_features, q_proj_weight, k_proj_weight, v_proj_weight,
                 o_proj_weight, token_positions):
    x = np.asarray(in_features, dtype=np.float32)
    wq = np.asarray(q_proj_weight, dtype=np.float32)
    wk = np.asarray(k_proj_weight, dtype=np.float32)
    wv = np.asarray(v_proj_weight, dtype=np.float32)
    wo = np.asarray(o_proj_weight, dtype=np.float32)
    csb, snb, psw, iden = _tables(token_positions)

    in_maps = []
    for c in range(NCORES):
        b, g = c // 4, c % 4
        hb = c // 4  # pair half index: cores 0-3 upload first halves
        r0 = g * CW + hb * 128
        in_maps.append({
            "x_sh": x[b, g * SQ:(g + 1) * SQ, :],
            "wq_h": wq[r0:r0 + 128, :],
            "wk_h": wk[r0:r0 + 128, :],
            "wv_h": wv[r0:r0 + 128, :],
            "wo_sh": wo[c * 128:(c + 1) * 128, :],
            "csb": csb,
            "snb": snb,
            "psw": psw,
            "iden": iden,
        })
    return in_maps


_NC_CACHE = []
last_exec_ns = None
last_res = None


def kernel(in_features, q_proj_weight, k_proj_weight, v_proj_weight,
           o_proj_weight, token_positions, d_model=1024, num_heads=16,
           **_ignored):
    global last_exec_ns, last_res
    assert int(d_model) == D and int(num_heads) == H
    in_maps = make_in_maps(in_features, q_proj_weight, k_proj_weight,
                           v_proj_weight, o_proj_weight, token_positions)
    if not _NC_CACHE:
        _NC_CACHE.append(build_nc())
    nc = _NC_CACHE[0]
    trace = bool(int(os.environ.get("KERNEL_TRACE", "0")))
    res = bass_utils.run_bass_kernel_spmd(nc, in_maps,
                                          core_ids=list(range(NCORES)),
                                          trace=trace)
    last_exec_ns = res.exec_time_ns
    last_res = res
    out = np.empty((B, S, D), np.float32)
    for c in range(NCORES):
        b, k = c // 4, c % 4
        p = res.results[c]["out"].astype(np.float32)
        for j in range(NSC):
            out[b, j * SQ + k * 128: j * SQ + (k + 1) * 128] = \
                p[j * 128:(j + 1) * 128]
    return out


# revision 29
# speedup vs baseline: 1.0477x; 1.0477x over previous
"""Trainium2 Bass kernel: single-layer causal attention block (q/k/v/o + RoPE).

v2 design — minimize host work + host<->device traffic, bf16 device compute:
  Sharding: 8 cores = 2 batches x 4 head-groups (4 heads each).
  Host uploads per core are CONTIGUOUS VIEWS of the original arrays only
  (plus tiny RoPE tables): the S/4 slice of x[b], half of the head-group's
  q/k/v weight rows (pair core has the other half), 128 rows of o_proj.
  On device:
    - AllGather(4) assembles full x[b]; AllGather(2) the q/k/v weight
      slices; AllToAll(8) deals each core its o_proj column block.
    - PE-transposes (+ f32->bf16 cast on PSUM copy-out) build x^T and
      W^T in SBUF.
    - RoPE in interleaved layout via a pair-swap permutation matmul and
      sign-folded cos/sin tables (no even/odd weight permutation).
    - scores^T = K^T-stationary bf16 matmuls (64-partition lhsT, one mm
      per key tile), exp on ACT with scale=1/8, multiplicative causal
      mask (built on-device via affine_select) on diagonal tiles,
      PV with a ones-column in V for the softmax row sums.
    - o_proj partials per 512-row chunk, ReduceScatter(add, 4) sums the
      head-group partials on device; each core outputs a [512, 1024] f32
      slice of the final output.
  Host gathers 8 x [512,1024] f32 pieces into [2, 2048, 1024].
"""

import os
import sys

import numpy as np

sys.path.insert(0, "/opt/trn_rl_repo")

import concourse.bass as bass  # noqa: E402
import concourse.tile as tile  # noqa: E402
from concourse import bacc, mybir  # noqa: E402
from concourse import bass_utils  # noqa: E402

B, S, D, H, DK = 2, 2048, 1024, 16, 64
NCORES = 8
HPC = H // 4     # 4 heads per core
CW = HPC * DK    # 256 head dims per core
VW = DK + 1      # 65: v width per head incl ones column
ND = D // 128    # 8 contraction chunks
NS = S // 128    # 16 s-tiles
NSC = S // 512   # 4 s-chunks
SQ = 512         # s-chunk size
ROPE_THETA = 10000.0

F32 = mybir.dt.float32
BF16 = mybir.dt.bfloat16
EXP = mybir.ActivationFunctionType.Exp

G4 = [[0, 1, 2, 3], [4, 5, 6, 7]]
G2 = [[0, 4], [1, 5], [2, 6], [3, 7]]
G8 = [[0, 1, 2, 3, 4, 5, 6, 7]]


def _build_kernel(tc, nc, t, dbg=False):
    from contextlib import ExitStack
    stack = ExitStack()

    csb, snb, psw = t["csb"], t["snb"], t["psw"]
    out = t["out"]
    xg, qkvg, woout = t["xg"], t["qkvg"], t["woout"]
    part, rsout = t["part"], t["rsout"]

    if not dbg:
        x_sh, wq_h, wk_h, wv_h, wo_sh = (t["x_sh"], t["wq_h"], t["wk_h"],
                                         t["wv_h"], t["wo_sh"])
        xcc, qkvcc, woin, wobh = t["xcc"], t["qkvcc"], t["woin"], t["wobh"]
        # ---- Phase A: cast inputs to bf16 (halves collective wire) ----
        with tc.tile_pool(name="precast", bufs=4) as pcp:
            for i in range(4):
                pa = pcp.tile([128, D], F32, name=f"pa{i}", tag="pa")
                eng = nc.sync if i % 2 == 0 else nc.scalar
                eng.dma_start(pa[:], x_sh[i * 128:(i + 1) * 128, :])
                pb = pcp.tile([128, D], BF16, name=f"pb{i}", tag="pb")
                nc.vector.tensor_copy(pb[:], pa[:])
                eng.dma_start(xcc[i * 128:(i + 1) * 128, :], pb[:])
            for i, wsrc in enumerate((wq_h, wk_h, wv_h, wo_sh)):
                pa = pcp.tile([128, D], F32, name=f"pw{i}", tag="pa")
                eng = nc.sync if i % 2 == 0 else nc.scalar
                eng.dma_start(pa[:], wsrc[:])
                pb = pcp.tile([128, D], BF16, name=f"pq{i}", tag="pb")
                nc.vector.tensor_copy(pb[:], pa[:])
                if i < 3:
                    eng.dma_start(qkvcc[i * 128:(i + 1) * 128, :], pb[:])
                else:
                    eng.dma_start(wobh[:], pb[:])
        # o_proj AllToAll input: shard j (rows j*128..) = wo_sh[:, (j%4)*256..]
        # so that core j receives o_proj[:, (j%4)*256:(j%4+1)*256] stacked
        # over all 8 contributors' row blocks.
        for j in range(8):
            nc.sync.dma_start(woin[j * 128:(j + 1) * 128, :],
                              wobh[:, (j % 4) * CW:(j % 4 + 1) * CW])
        nc.gpsimd.collective_compute(
            "AllGather", mybir.AluOpType.bypass, replica_groups=G4,
            ins=[xcc[:]], outs=[xg[:]])
        nc.gpsimd.collective_compute(
            "AllGather", mybir.AluOpType.bypass, replica_groups=G2,
            ins=[qkvcc[:]], outs=[qkvg[:]])
        nc.gpsimd.collective_compute(
            "AllToAll", mybir.AluOpType.bypass, replica_groups=G8,
            ins=[woin[:]], outs=[woout[:]])

    constp = stack.enter_context(tc.tile_pool(name="const", bufs=1))
    pers = stack.enter_context(tc.tile_pool(name="persist", bufs=1))

    # ---- constants: RoPE tables, pair-swap matrix, causal masks ----
    cs_sb = constp.tile([128, S], BF16)
    sn_sb = constp.tile([128, S], BF16)
    nc.sync.dma_start(cs_sb[0:64, :], csb[:])
    nc.sync.dma_start(cs_sb[64:128, :], csb[:])
    nc.sync.dma_start(sn_sb[0:64, :], snb[:])
    nc.sync.dma_start(sn_sb[64:128, :], snb[:])
    psw_sb = constp.tile([128, 128], BF16)
    nc.sync.dma_start(psw_sb[:], psw[:])
    one_sb = constp.tile([128, SQ], BF16)
    nc.gpsimd.memset(one_sb[:], 1.0)
    mk_sb = constp.tile([128, NSC * SQ], BF16)
    for j in range(NSC):
        # keep 1.0 where free_idx - partition - 128*j >= 0, else 0
        nc.gpsimd.affine_select(
            mk_sb[:, j * SQ:(j + 1) * SQ], one_sb[:],
            pattern=[[1, SQ]], compare_op=mybir.AluOpType.is_ge,
            fill=0.0, base=-(128 * j), channel_multiplier=-1)

    # persistent tensors
    qrot = [pers.tile([128, S], BF16, name=f"qrot{i}") for i in range(2)]
    krot = [pers.tile([128, S], BF16, name=f"krot{i}") for i in range(2)]
    v_sb = pers.tile([128, NS * HPC * VW], BF16)
    ctx_sb = pers.tile([128, 2 * S], BF16)
    wo_sb = pers.tile([128, 2 * D], BF16)
    nc.gpsimd.memset(v_sb[:], 1.0)  # pre-fill the per-head ones columns

    # ---- Phase B: load f32 tiles, PE-transpose, cast-copy to bf16 SBUF ----
    id_sb = constp.tile([128, 128], BF16)
    nc.sync.dma_start(id_sb[:], t["iden"][:])
    xtp = stack.enter_context(tc.tile_pool(name="xt", bufs=1))
    xt_sb = xtp.tile([128, ND * S], BF16)
    wtp2 = stack.enter_context(tc.tile_pool(name="wt", bufs=1))
    wT = [wtp2.tile([128, ND * CW], BF16, name=f"wT{w}") for w in range(3)]
    with tc.tile_pool(name="xload", bufs=3) as xlp, \
         tc.tile_pool(name="tps", bufs=4, space="PSUM") as tpp:
        for st in range(NS):
            xa = xlp.tile([128, D], BF16, name=f"xa{st}", tag="xa")
            eng = nc.sync if st % 2 == 0 else nc.scalar
            eng.dma_start(xa[:], xg[st * 128:(st + 1) * 128, :])
            for d in range(ND):
                pt = tpp.tile([128, 128], BF16, name=f"xt{st}_{d}", tag="tp")
                nc.tensor.transpose(pt[:], xa[:, d * 128:(d + 1) * 128],
                                    id_sb[:])
                dst = xt_sb[:, d * S + st * 128: d * S + (st + 1) * 128]
                if d % 2 == 0:
                    nc.vector.tensor_copy(dst, pt[:])
                else:
                    nc.scalar.copy(dst, pt[:])

        # ---- weights after AG2: same PE-transpose path ----
        for blk in range(6):
            w, half = blk % 3, blk // 3
            src_r = half * 384 + w * 128
            wa = xlp.tile([128, D], BF16, name=f"wa{blk}", tag="xa")
            eng = nc.sync if blk % 2 == 0 else nc.scalar
            eng.dma_start(wa[:], qkvg[src_r:src_r + 128, :])
            for d in range(ND):
                pt = tpp.tile([128, 128], BF16, name=f"wt{blk}_{d}", tag="tp")
                nc.tensor.transpose(pt[:], wa[:, d * 128:(d + 1) * 128],
                                    id_sb[:])
                dst = wT[w][:, d * CW + half * 128: d * CW + half * 128 + 128]
                if d % 2 == 0:
                    nc.vector.tensor_copy(dst, pt[:])
                else:
                    nc.scalar.copy(dst, pt[:])

        # ---- o_proj weight after AllToAll ----
        for jt in range(ND):
            oa = xlp.tile([128, CW], BF16, name=f"oa{jt}", tag="xa")
            eng = nc.sync if jt % 2 == 0 else nc.scalar
            eng.dma_start(oa[:], woout[jt * 128:(jt + 1) * 128, :])
            for cb in range(2):
                pt = tpp.tile([128, 128], BF16, name=f"ot{jt}_{cb}", tag="tp")
                nc.tensor.transpose(pt[:], oa[:, cb * 128:(cb + 1) * 128],
                                    id_sb[:])
                dst = wo_sb[:, cb * D + jt * 128: cb * D + (jt + 1) * 128]
                if jt % 2 == 0:
                    nc.vector.tensor_copy(dst, pt[:])
                else:
                    nc.scalar.copy(dst, pt[:])

    # ---- Phase C: projections (bf16 matmuls, f32 psum) + RoPE ----
    with tc.tile_pool(name="qraw", bufs=1) as qrp, \
         tc.tile_pool(name="pjps", bufs=4, space="PSUM") as pjps, \
         tc.tile_pool(name="swps", bufs=2, space="PSUM") as swps, \
         tc.tile_pool(name="vps", bufs=2, space="PSUM") as vps, \
         tc.tile_pool(name="ropet", bufs=4) as rtp:
        blocks = [(qrot[0], wT[0], 0), (qrot[1], wT[0], 128),
                  (krot[0], wT[1], 0), (krot[1], wT[1], 128)]
        qraw = [qrp.tile([128, S], BF16, name=f"qraw{i}") for i in range(4)]
        for bi, (dst, wsb, co) in enumerate(blocks):
            raw = qraw[bi]
            for sc in range(NSC):
                ps = pjps.tile([128, SQ], F32, name=f"pj{bi}_{sc}", tag="pj")
                for d in range(ND):
                    nc.tensor.matmul(
                        ps[:],
                        wsb[:, d * CW + co: d * CW + co + 128],
                        xt_sb[:, d * S + sc * SQ: d * S + (sc + 1) * SQ],
                        start=(d == 0), stop=(d == ND - 1))
                nc.scalar.copy(raw[:, sc * SQ:(sc + 1) * SQ], ps[:])
                # pair-swap via permutation matmul, then sign-folded rope
                psx = swps.tile([128, SQ], F32, name=f"sw{bi}_{sc}", tag="sw")
                nc.tensor.matmul(psx[:], psw_sb[:],
                                 raw[:, sc * SQ:(sc + 1) * SQ],
                                 start=True, stop=True)
                t1 = rtp.tile([128, SQ], BF16, name=f"t1_{bi}_{sc}", tag="t1")
                t2 = rtp.tile([128, SQ], BF16, name=f"t2_{bi}_{sc}", tag="t2")
                nc.vector.tensor_mul(t1[:], raw[:, sc * SQ:(sc + 1) * SQ],
                                     cs_sb[:, sc * SQ:(sc + 1) * SQ])
                nc.vector.tensor_mul(t2[:], psx[:],
                                     sn_sb[:, sc * SQ:(sc + 1) * SQ])
                nc.vector.tensor_add(dst[:, sc * SQ:(sc + 1) * SQ], t1[:], t2[:])
        for sm in range(NS):
            pv = vps.tile([128, CW], F32, name=f"pv{sm}", tag="pv")
            for d in range(ND):
                nc.tensor.matmul(
                    pv[:],
                    xt_sb[:, d * S + sm * 128: d * S + sm * 128 + 128],
                    wT[2][:, d * CW:(d + 1) * CW],
                    start=(d == 0), stop=(d == ND - 1))
            base = sm * HPC * VW
            dst3 = v_sb[:, base:base + HPC * VW].rearrange("p (h c) -> p h c", c=VW)
            nc.vector.tensor_copy(dst3[:, :, 0:DK],
                                  pv[:].rearrange("p (h c) -> p h c", c=DK))

    # ---- Phase D: attention (c-outer, h-inner) + o_proj + chunked RS ----
    with tc.tile_pool(name="sps", bufs=4, space="PSUM") as sps, \
         tc.tile_pool(name="cps", bufs=2, space="PSUM") as cps, \
         tc.tile_pool(name="ops", bufs=2, space="PSUM") as opsp, \
         tc.tile_pool(name="expool", bufs=6) as exp_pool, \
         tc.tile_pool(name="smp", bufs=8) as smp, \
         tc.tile_pool(name="obuf", bufs=4) as obp:
        for c in range(NSC):
            nsk = 4 * (c + 1)
            for h in range(HPC):
                T = h // 2
                r0 = (h % 2) * 64
                tp = (r0, 0)
                pctx = cps.tile([VW, SQ], F32, name=f"pc{c}_{h}", tag="pc")
                exps = []
                DEPTH = 3

                def pv_mm(ti, nsk=nsk, pctx=pctx, exps=exps, h=h):
                    vbase = ti * HPC * VW + h * VW
                    nc.tensor.matmul(
                        pctx[:],
                        v_sb[:, vbase:vbase + VW],
                        exps[ti][:],
                        start=(ti == 0), stop=(ti == nsk - 1),
                        skip_group_check=True)

                for ti in range(nsk):
                    pscore = sps.tile([128, SQ], F32, name=f"ps{c}_{h}_{ti}",
                                      tag="ps")
                    nc.tensor.matmul(
                        pscore[:],
                        krot[T][r0:r0 + 64, ti * 128:(ti + 1) * 128],
                        qrot[T][r0:r0 + 64, c * SQ:(c + 1) * SQ],
                        start=True, stop=True, tile_position=tp,
                        skip_group_check=True)
                    et = exp_pool.tile([128, SQ], BF16, name=f"e{c}_{h}_{ti}",
                                       tag="et")
                    nc.scalar.activation(et[:], pscore[:], EXP, scale=0.125)
                    j = ti - 4 * c
                    if j >= 0:
                        nc.vector.tensor_mul(et[:], et[:],
                                             mk_sb[:, j * SQ:(j + 1) * SQ])
                    exps.append(et)
                    if ti >= DEPTH:
                        pv_mm(ti - DEPTH)
                for ti in range(max(0, nsk - DEPTH), nsk):
                    pv_mm(ti)
                rs0 = smp.tile([1, SQ], F32, name=f"rs{c}_{h}", tag="rs0")
                nc.vector.tensor_copy(rs0[:], pctx[DK:DK + 1, :])
                rc = smp.tile([1, SQ], F32, name=f"rc{c}_{h}", tag="rc")
                nc.vector.reciprocal_approx_fast(rc[:], rs0[:])
                rb = smp.tile([64, SQ], F32, name=f"rb{c}_{h}", tag="rb")
                nc.gpsimd.partition_broadcast(rb[:], rc[:])
                nc.vector.tensor_mul(
                    ctx_sb[r0:r0 + 64,
                           (h // 2) * S + c * SQ:(h // 2) * S + (c + 1) * SQ],
                    pctx[0:DK, :], rb[:])
            if not dbg and c > 0:
                cp = c - 1
                nc.gpsimd.collective_compute(
                    "ReduceScatter", mybir.AluOpType.add, replica_groups=G4,
                    ins=[part[cp * SQ:(cp + 1) * SQ, :]],
                    outs=[rsout[cp * 128:(cp + 1) * 128, :]])
            # o_proj for this 512-row chunk, then ReduceScatter it
            for sm in range(4 * c, 4 * c + 4):
                pos = [opsp.tile([128, SQ], F32, name=f"op{sm}_{i}", tag="op")
                       for i in range(2)]
                for cb in range(2):
                    lhsT = ctx_sb[:, cb * S + sm * 128: cb * S + sm * 128 + 128]
                    for do_ in range(2):
                        nc.tensor.matmul(
                            pos[do_][:],
                            lhsT,
                            wo_sb[:, cb * D + do_ * SQ: cb * D + (do_ + 1) * SQ],
                            start=(cb == 0), stop=(cb == 1))
                ot = obp.tile([128, D], BF16, name=f"ot{sm}", tag="ot")
                nc.vector.tensor_copy(ot[:, 0:SQ], pos[0][:])
                nc.vector.tensor_copy(ot[:, SQ:D], pos[1][:])
                nc.sync.dma_start(part[sm * 128:(sm + 1) * 128, :], ot[:])
            if not dbg and c == NSC - 1:
                nc.gpsimd.collective_compute(
                    "ReduceScatter", mybir.AluOpType.add, replica_groups=G4,
                    ins=[part[c * SQ:(c + 1) * SQ, :]],
                    outs=[rsout[c * 128:(c + 1) * 128, :]])

        if not dbg:
            # ---- output: upcast rsout chunks to f32 and store ----
            with tc.tile_pool(name="oq", bufs=4) as oqp:
                for j in range(NSC):
                    rt = oqp.tile([128, D], BF16, name=f"rt{j}", tag="rt")
                    nc.sync.dma_start(rt[:], rsout[j * 128:(j + 1) * 128, :])
                    rf = oqp.tile([128, D], F32, name=f"rf{j}", tag="rf")
                    nc.vector.tensor_copy(rf[:], rt[:])
                    nc.sync.dma_start(out[j * 128:(j + 1) * 128, :], rf[:])
        else:
            nc.sync.dma_start(t["dbg_xt"][:], xt_sb[:])
            nc.sync.dma_start(t["dbg_q"][:], qrot[0][:])
            nc.sync.dma_start(t["dbg_k"][:], krot[0][:])
            nc.sync.dma_start(t["dbg_v"][:], v_sb[:])
            nc.sync.dma_start(t["dbg_ctx"][:], ctx_sb[:])
            nc.sync.dma_start(t["dbg_wo"][:], wo_sb[:])
    stack.close()


def build_nc(dbg=False):
    nc = bacc.Bacc("TRN2", target_bir_lowering=False, debug=False,
                   enable_asserts=False, num_devices=NCORES)
    t = {}
    if not dbg:
        t["x_sh"] = nc.dram_tensor("x_sh", [SQ, D], F32, kind="ExternalInput").ap()
        t["wq_h"] = nc.dram_tensor("wq_h", [128, D], F32, kind="ExternalInput").ap()
        t["wk_h"] = nc.dram_tensor("wk_h", [128, D], F32, kind="ExternalInput").ap()
        t["wv_h"] = nc.dram_tensor("wv_h", [128, D], F32, kind="ExternalInput").ap()
        t["wo_sh"] = nc.dram_tensor("wo_sh", [128, D], F32,
                                    kind="ExternalInput").ap()
    t["csb"] = nc.dram_tensor("csb", [64, S], BF16, kind="ExternalInput").ap()
    t["snb"] = nc.dram_tensor("snb", [64, S], BF16, kind="ExternalInput").ap()
    t["psw"] = nc.dram_tensor("psw", [128, 128], BF16, kind="ExternalInput").ap()
    t["iden"] = nc.dram_tensor("iden", [128, 128], BF16, kind="ExternalInput").ap()
    t["out"] = nc.dram_tensor("out", [SQ, D], F32, kind="ExternalOutput").ap()

    ikind = "ExternalInput" if dbg else "Internal"
    t["xg"] = nc.dram_tensor("xg", [S, D], BF16, kind=ikind).ap()
    t["qkvg"] = nc.dram_tensor("qkvg", [768, D], BF16, kind=ikind).ap()
    t["woout"] = nc.dram_tensor("woout", [D, CW], BF16, kind=ikind).ap()
    if not dbg:
        t["xcc"] = nc.dram_tensor("xcc", [SQ, D], BF16, kind="Internal").ap()
        t["qkvcc"] = nc.dram_tensor("qkvcc", [384, D], BF16, kind="Internal").ap()
        t["woin"] = nc.dram_tensor("woin", [D, CW], BF16, kind="Internal").ap()
        t["wobh"] = nc.dram_tensor("wobh", [128, D], BF16, kind="Internal").ap()
    t["part"] = nc.dram_tensor("part", [S, D], BF16,
                               kind="ExternalOutput" if dbg else "Internal").ap()
    t["rsout"] = nc.dram_tensor("rsout", [SQ, D], BF16, kind="Internal").ap()
    if dbg:
        t["dbg_xt"] = nc.dram_tensor("dbg_xt", [128, ND * S], BF16,
                                     kind="ExternalOutput").ap()
        t["dbg_q"] = nc.dram_tensor("dbg_q", [128, S], BF16,
                                    kind="ExternalOutput").ap()
        t["dbg_k"] = nc.dram_tensor("dbg_k", [128, S], BF16,
                                    kind="ExternalOutput").ap()
        t["dbg_v"] = nc.dram_tensor("dbg_v", [128, NS * HPC * VW], BF16,
                                    kind="ExternalOutput").ap()
        t["dbg_ctx"] = nc.dram_tensor("dbg_ctx", [128, 2 * S], BF16,
                                      kind="ExternalOutput").ap()
        t["dbg_wo"] = nc.dram_tensor("dbg_wo", [128, 2 * D], BF16,
                                     kind="ExternalOutput").ap()
    with tile.TileContext(nc) as tc:
        _build_kernel(tc, nc, t, dbg=dbg)
    nc.compile()
    return nc


_TABLES = []


def _tables(token_positions):
    if _TABLES:
        return _TABLES[0]
    import ml_dtypes
    pos = np.asarray(token_positions).astype(np.float64)
    inv = ROPE_THETA ** (-2.0 * np.arange(DK // 2, dtype=np.float64) / DK)
    ang = inv[:, None] * pos[None, :]                    # [32, S]
    cs = np.repeat(np.cos(ang), 2, axis=0)               # [64, S]
    sn = np.repeat(np.sin(ang), 2, axis=0)
    sn[0::2] *= -1.0                                     # sign-folded
    csb = cs.astype(ml_dtypes.bfloat16)
    snb = sn.astype(ml_dtypes.bfloat16)
    idx = np.arange(128)
    psw = (idx[:, None] == (idx[None, :] ^ 1)).astype(ml_dtypes.bfloat16)
    iden = np.eye(128, dtype=ml_dtypes.bfloat16)
    _TABLES.append((csb, snb, psw, iden))
    return _TABLES[0]


def make_in_maps(in_features, q_proj_weight, k_proj_weight, v_proj_weight,
                 o_proj_weight, token_positions):
    x = np.asarray(in_features, dtype=np.float32)
    wq = np.asarray(q_proj_weight, dtype=np.float32)
    wk = np.asarray(k_proj_weight, dtype=np.float32)
    wv = np.asarray(v_proj_weight, dtype=np.float32)
    wo = np.asarray(o_proj_weight, dtype=np.float32)
    csb, snb, psw, iden = _tables(token_positions)

    in_maps = []
    for c in range(NCORES):
        b, g = c // 4, c % 4
        hb = c // 4  # pair half index: cores 0-3 upload first halves
        r0 = g * CW + hb * 128
        in_maps.append({
            "x_sh": x[b, g * SQ:(g + 1) * SQ, :],
            "wq_h": wq[r0:r0 + 128, :],
            "wk_h": wk[r0:r0 + 128, :],
            "wv_h": wv[r0:r0 + 128, :],
            "wo_sh": wo[c * 128:(c + 1) * 128, :],
            "csb": csb,
            "snb": snb,
            "psw": psw,
            "iden": iden,
        })
    return in_maps


_NC_CACHE = []
last_exec_ns = None
last_res = None


def kernel(in_features, q_proj_weight, k_proj_weight, v_proj_weight,
           o_proj_weight, token_positions, d_model=1024, num_heads=16,
           **_ignored):
    global last_exec_ns, last_res
    assert int(d_model) == D and int(num_heads) == H
    in_maps = make_in_maps(in_features, q_proj_weight, k_proj_weight,
                           v_proj_weight, o_proj_weight, token_positions)
    if not _NC_CACHE:
        _NC_CACHE.append(build_nc())
    nc = _NC_CACHE[0]
    trace = bool(int(os.environ.get("KERNEL_TRACE", "0")))
    res = bass_utils.run_bass_kernel_spmd(nc, in_maps,
                                          core_ids=list(range(NCORES)),
                                          trace=trace)
    last_exec_ns = res.exec_time_ns
    last_res = res
    out = np.empty((B, S, D), np.float32)
    for c in range(NCORES):
        b, k = c // 4, c % 4
        p = res.results[c]["out"].astype(np.float32)
        for j in range(NSC):
            out[b, j * SQ + k * 128: j * SQ + (k + 1) * 128] = \
                p[j * 128:(j + 1) * 128]
    return out


# revision 31
# speedup vs baseline: 2942.6445x; 2808.5408x over previous
"""Trainium2 Bass kernel: single-layer causal attention block (q/k/v/o + RoPE).

v2 design — minimize host work + host<->device traffic, bf16 device compute:
  Sharding: 8 cores = 2 batches x 4 head-groups (4 heads each).
  Host uploads per core are CONTIGUOUS VIEWS of the original arrays only
  (plus tiny RoPE tables): the S/4 slice of x[b], half of the head-group's
  q/k/v weight rows (pair core has the other half), 128 rows of o_proj.
  On device:
    - AllGather(4) assembles full x[b]; AllGather(2) the q/k/v weight
      slices; AllToAll(8) deals each core its o_proj column block.
    - PE-transposes (+ f32->bf16 cast on PSUM copy-out) build x^T and
      W^T in SBUF.
    - RoPE in interleaved layout via a pair-swap permutation matmul and
      sign-folded cos/sin tables (no even/odd weight permutation).
    - scores^T = K^T-stationary bf16 matmuls (64-partition lhsT, one mm
      per key tile), exp on ACT with scale=1/8, multiplicative causal
      mask (built on-device via affine_select) on diagonal tiles,
      PV with a ones-column in V for the softmax row sums.
    - o_proj partials per 512-row chunk, ReduceScatter(add, 4) sums the
      head-group partials on device; each core outputs a [512, 1024] f32
      slice of the final output.
  Host gathers 8 x [512,1024] f32 pieces into [2, 2048, 1024].
"""

import os
import sys

import numpy as np

sys.path.insert(0, "/opt/trn_rl_repo")

import concourse.bass as bass  # noqa: E402
import concourse.tile as tile  # noqa: E402
from concourse import bacc, mybir  # noqa: E402
from concourse import bass_utils  # noqa: E402

B, S, D, H, DK = 2, 2048, 1024, 16, 64
NCORES = 8
HPC = H // 4     # 4 heads per core
CW = HPC * DK    # 256 head dims per core
VW = DK + 1      # 65: v width per head incl ones column
ND = D // 128    # 8 contraction chunks
NS = S // 128    # 16 s-tiles
NSC = S // 512   # 4 s-chunks
SQ = 512         # s-chunk size
ROPE_THETA = 10000.0

F32 = mybir.dt.float32
BF16 = mybir.dt.bfloat16
EXP = mybir.ActivationFunctionType.Exp

G4 = [[0, 1, 2, 3], [4, 5, 6, 7]]
G2 = [[0, 4], [1, 5], [2, 6], [3, 7]]
G8 = [[0, 1, 2, 3, 4, 5, 6, 7]]


def _build_kernel(tc, nc, t, dbg=False):
    from contextlib import ExitStack
    stack = ExitStack()

    csb, snb, psw = t["csb"], t["snb"], t["psw"]
    out = t["out"]
    xg, qkvg, woout = t["xg"], t["qkvg"], t["woout"]
    part, rsout = t["part"], t["rsout"]

    if not dbg:
        x_sh, wq_h, wk_h, wv_h, wo_sh = (t["x_sh"], t["wq_h"], t["wk_h"],
                                         t["wv_h"], t["wo_sh"])
        xcc, qkvcc, woin, wobh = t["xcc"], t["qkvcc"], t["woin"], t["wobh"]
        # ---- Phase A: cast inputs to bf16 (halves collective wire) ----
        with tc.tile_pool(name="precast", bufs=4) as pcp:
            for i in range(4):
                pa = pcp.tile([128, D], F32, name=f"pa{i}", tag="pa")
                eng = nc.sync if i % 2 == 0 else nc.scalar
                eng.dma_start(pa[:], x_sh[i * 128:(i + 1) * 128, :])
                pb = pcp.tile([128, D], BF16, name=f"pb{i}", tag="pb")
                nc.vector.tensor_copy(pb[:], pa[:])
                eng.dma_start(xcc[i * 128:(i + 1) * 128, :], pb[:])
            for i, wsrc in enumerate((wq_h, wk_h, wv_h, wo_sh)):
                pa = pcp.tile([128, D], F32, name=f"pw{i}", tag="pa")
                eng = nc.sync if i % 2 == 0 else nc.scalar
                eng.dma_start(pa[:], wsrc[:])
                pb = pcp.tile([128, D], BF16, name=f"pq{i}", tag="pb")
                nc.vector.tensor_copy(pb[:], pa[:])
                if i < 3:
                    eng.dma_start(qkvcc[i * 128:(i + 1) * 128, :], pb[:])
                else:
                    eng.dma_start(wobh[:], pb[:])
        # o_proj AllToAll input: shard j (rows j*128..) = wo_sh[:, (j%4)*256..]
        # so that core j receives o_proj[:, (j%4)*256:(j%4+1)*256] stacked
        # over all 8 contributors' row blocks.
        for j in range(8):
            nc.sync.dma_start(woin[j * 128:(j + 1) * 128, :],
                              wobh[:, (j % 4) * CW:(j % 4 + 1) * CW])
        nc.gpsimd.collective_compute(
            "AllGather", mybir.AluOpType.bypass, replica_groups=G4,
            ins=[xcc[0:256, :]], outs=[xg[0:1024, :]])
        nc.gpsimd.collective_compute(
            "AllGather", mybir.AluOpType.bypass, replica_groups=G4,
            ins=[xcc[256:512, :]], outs=[xg[1024:2048, :]])
        nc.gpsimd.collective_compute(
            "AllGather", mybir.AluOpType.bypass, replica_groups=G2,
            ins=[qkvcc[:]], outs=[qkvg[:]])
        nc.gpsimd.collective_compute(
            "AllToAll", mybir.AluOpType.bypass, replica_groups=G8,
            ins=[woin[:]], outs=[woout[:]])

    constp = stack.enter_context(tc.tile_pool(name="const", bufs=1))
    pers = stack.enter_context(tc.tile_pool(name="persist", bufs=1))

    # ---- constants: RoPE tables, pair-swap matrix, causal masks ----
    cs_sb = constp.tile([128, S], BF16)
    sn_sb = constp.tile([128, S], BF16)
    nc.sync.dma_start(cs_sb[0:64, :], csb[:])
    nc.sync.dma_start(cs_sb[64:128, :], csb[:])
    nc.sync.dma_start(sn_sb[0:64, :], snb[:])
    nc.sync.dma_start(sn_sb[64:128, :], snb[:])
    psw_sb = constp.tile([128, 128], BF16)
    nc.sync.dma_start(psw_sb[:], psw[:])
    one_sb = constp.tile([128, SQ], BF16)
    nc.gpsimd.memset(one_sb[:], 1.0)
    mk_sb = constp.tile([128, NSC * SQ], BF16)
    for j in range(NSC):
        # keep 1.0 where free_idx - partition - 128*j >= 0, else 0
        nc.gpsimd.affine_select(
            mk_sb[:, j * SQ:(j + 1) * SQ], one_sb[:],
            pattern=[[1, SQ]], compare_op=mybir.AluOpType.is_ge,
            fill=0.0, base=-(128 * j), channel_multiplier=-1)

    # persistent tensors
    qrot = [pers.tile([128, S], BF16, name=f"qrot{i}") for i in range(2)]
    krot = [pers.tile([128, S], BF16, name=f"krot{i}") for i in range(2)]
    v_sb = pers.tile([128, NS * HPC * VW], BF16)
    ctx_sb = pers.tile([128, 2 * S], BF16)
    wo_sb = pers.tile([128, 2 * D], BF16)
    nc.gpsimd.memset(v_sb[:], 1.0)  # pre-fill the per-head ones columns

    # ---- Phase B: load f32 tiles, PE-transpose, cast-copy to bf16 SBUF ----
    id_sb = constp.tile([128, 128], BF16)
    nc.sync.dma_start(id_sb[:], t["iden"][:])
    xtp = stack.enter_context(tc.tile_pool(name="xt", bufs=1))
    xt_sb = xtp.tile([128, ND * S], BF16)
    wtp2 = stack.enter_context(tc.tile_pool(name="wt", bufs=1))
    wT = [wtp2.tile([128, ND * CW], BF16, name=f"wT{w}") for w in range(3)]
    with tc.tile_pool(name="xload", bufs=3) as xlp, \
         tc.tile_pool(name="tps", bufs=4, space="PSUM") as tpp:
        for st in range(NS):
            xa = xlp.tile([128, D], BF16, name=f"xa{st}", tag="xa")
            eng = nc.sync if st % 2 == 0 else nc.scalar
            g_, r_ = st // 4, st % 4
            src_row = (r_ // 2) * 1024 + g_ * 256 + (r_ % 2) * 128
            eng.dma_start(xa[:], xg[src_row:src_row + 128, :])
            for d in range(ND):
                pt = tpp.tile([128, 128], BF16, name=f"xt{st}_{d}", tag="tp")
                nc.tensor.transpose(pt[:], xa[:, d * 128:(d + 1) * 128],
                                    id_sb[:])
                dst = xt_sb[:, d * S + st * 128: d * S + (st + 1) * 128]
                if d % 2 == 0:
                    nc.vector.tensor_copy(dst, pt[:])
                else:
                    nc.scalar.copy(dst, pt[:])

        # ---- weights after AG2: same PE-transpose path ----
        for blk in range(6):
            w, half = blk % 3, blk // 3
            src_r = half * 384 + w * 128
            wa = xlp.tile([128, D], BF16, name=f"wa{blk}", tag="xa")
            eng = nc.sync if blk % 2 == 0 else nc.scalar
            eng.dma_start(wa[:], qkvg[src_r:src_r + 128, :])
            for d in range(ND):
                pt = tpp.tile([128, 128], BF16, name=f"wt{blk}_{d}", tag="tp")
                nc.tensor.transpose(pt[:], wa[:, d * 128:(d + 1) * 128],
                                    id_sb[:])
                dst = wT[w][:, d * CW + half * 128: d * CW + half * 128 + 128]
                if d % 2 == 0:
                    nc.vector.tensor_copy(dst, pt[:])
                else:
                    nc.scalar.copy(dst, pt[:])

        # ---- o_proj weight after AllToAll ----
        for jt in range(ND):
            oa = xlp.tile([128, CW], BF16, name=f"oa{jt}", tag="xa")
            eng = nc.sync if jt % 2 == 0 else nc.scalar
            eng.dma_start(oa[:], woout[jt * 128:(jt + 1) * 128, :])
            for cb in range(2):
                pt = tpp.tile([128, 128], BF16, name=f"ot{jt}_{cb}", tag="tp")
                nc.tensor.transpose(pt[:], oa[:, cb * 128:(cb + 1) * 128],
                                    id_sb[:])
                dst = wo_sb[:, cb * D + jt * 128: cb * D + (jt + 1) * 128]
                if jt % 2 == 0:
                    nc.vector.tensor_copy(dst, pt[:])
                else:
                    nc.scalar.copy(dst, pt[:])

    # ---- Phase C: projections (bf16 matmuls, f32 psum) + RoPE ----
    with tc.tile_pool(name="qraw", bufs=1) as qrp, \
         tc.tile_pool(name="pjps", bufs=4, space="PSUM") as pjps, \
         tc.tile_pool(name="swps", bufs=2, space="PSUM") as swps, \
         tc.tile_pool(name="vps", bufs=2, space="PSUM") as vps, \
         tc.tile_pool(name="ropet", bufs=4) as rtp:
        blocks = [(qrot[0], wT[0], 0), (qrot[1], wT[0], 128),
                  (krot[0], wT[1], 0), (krot[1], wT[1], 128)]
        qraw = [qrp.tile([128, S], BF16, name=f"qraw{i}") for i in range(4)]
        for bi, (dst, wsb, co) in enumerate(blocks):
            raw = qraw[bi]
            for sc in range(NSC):
                ps = pjps.tile([128, SQ], F32, name=f"pj{bi}_{sc}", tag="pj")
                for d in range(ND):
                    nc.tensor.matmul(
                        ps[:],
                        wsb[:, d * CW + co: d * CW + co + 128],
                        xt_sb[:, d * S + sc * SQ: d * S + (sc + 1) * SQ],
                        start=(d == 0), stop=(d == ND - 1))
                nc.scalar.copy(raw[:, sc * SQ:(sc + 1) * SQ], ps[:])
                # pair-swap via permutation matmul, then sign-folded rope
                psx = swps.tile([128, SQ], F32, name=f"sw{bi}_{sc}", tag="sw")
                nc.tensor.matmul(psx[:], psw_sb[:],
                                 raw[:, sc * SQ:(sc + 1) * SQ],
                                 start=True, stop=True)
                t1 = rtp.tile([128, SQ], BF16, name=f"t1_{bi}_{sc}", tag="t1")
                t2 = rtp.tile([128, SQ], BF16, name=f"t2_{bi}_{sc}", tag="t2")
                nc.vector.tensor_mul(t1[:], raw[:, sc * SQ:(sc + 1) * SQ],
                                     cs_sb[:, sc * SQ:(sc + 1) * SQ])
                nc.vector.tensor_mul(t2[:], psx[:],
                                     sn_sb[:, sc * SQ:(sc + 1) * SQ])
                nc.vector.tensor_add(dst[:, sc * SQ:(sc + 1) * SQ], t1[:], t2[:])
        for sm in range(NS):
            pv = vps.tile([128, CW], F32, name=f"pv{sm}", tag="pv")
            for d in range(ND):
                nc.tensor.matmul(
                    pv[:],
                    xt_sb[:, d * S + sm * 128: d * S + sm * 128 + 128],
                    wT[2][:, d * CW:(d + 1) * CW],
                    start=(d == 0), stop=(d == ND - 1))
            base = sm * HPC * VW
            dst3 = v_sb[:, base:base + HPC * VW].rearrange("p (h c) -> p h c", c=VW)
            nc.vector.tensor_copy(dst3[:, :, 0:DK],
                                  pv[:].rearrange("p (h c) -> p h c", c=DK))

    # ---- Phase D: attention (c-outer, h-inner) + o_proj + chunked RS ----
    with tc.tile_pool(name="sps", bufs=4, space="PSUM") as sps, \
         tc.tile_pool(name="cps", bufs=2, space="PSUM") as cps, \
         tc.tile_pool(name="ops", bufs=2, space="PSUM") as opsp, \
         tc.tile_pool(name="expool", bufs=6) as exp_pool, \
         tc.tile_pool(name="smp", bufs=8) as smp, \
         tc.tile_pool(name="oup", bufs=2) as oup, \
         tc.tile_pool(name="obuf", bufs=4) as obp:
        for c in range(NSC):
            nsk = 4 * (c + 1)
            for h in range(HPC):
                T = h // 2
                r0 = (h % 2) * 64
                tp = (r0, 0)
                pctx = cps.tile([VW, SQ], F32, name=f"pc{c}_{h}", tag="pc")
                exps = []
                DEPTH = 3

                def pv_mm(ti, nsk=nsk, pctx=pctx, exps=exps, h=h):
                    vbase = ti * HPC * VW + h * VW
                    nc.tensor.matmul(
                        pctx[:],
                        v_sb[:, vbase:vbase + VW],
                        exps[ti][:],
                        start=(ti == 0), stop=(ti == nsk - 1),
                        skip_group_check=True)

                for ti in range(nsk):
                    pscore = sps.tile([128, SQ], F32, name=f"ps{c}_{h}_{ti}",
                                      tag="ps")
                    nc.tensor.matmul(
                        pscore[:],
                        krot[T][r0:r0 + 64, ti * 128:(ti + 1) * 128],
                        qrot[T][r0:r0 + 64, c * SQ:(c + 1) * SQ],
                        start=True, stop=True, tile_position=tp,
                        skip_group_check=True)
                    et = exp_pool.tile([128, SQ], BF16, name=f"e{c}_{h}_{ti}",
                                       tag="et")
                    nc.scalar.activation(et[:], pscore[:], EXP, scale=0.125)
                    j = ti - 4 * c
                    if j >= 0:
                        nc.vector.tensor_mul(et[:], et[:],
                                             mk_sb[:, j * SQ:(j + 1) * SQ])
                    exps.append(et)
                    if ti >= DEPTH:
                        pv_mm(ti - DEPTH)
                for ti in range(max(0, nsk - DEPTH), nsk):
                    pv_mm(ti)
                rs0 = smp.tile([1, SQ], F32, name=f"rs{c}_{h}", tag="rs0")
                nc.vector.tensor_copy(rs0[:], pctx[DK:DK + 1, :])
                rc = smp.tile([1, SQ], F32, name=f"rc{c}_{h}", tag="rc")
                nc.vector.reciprocal_approx_fast(rc[:], rs0[:])
                rb = smp.tile([64, SQ], F32, name=f"rb{c}_{h}", tag="rb")
                nc.gpsimd.partition_broadcast(rb[:], rc[:])
                nc.vector.tensor_mul(
                    ctx_sb[r0:r0 + 64,
                           (h // 2) * S + c * SQ:(h // 2) * S + (c + 1) * SQ],
                    pctx[0:DK, :], rb[:])
            if not dbg and c > 0:
                cp = c - 1
                nc.gpsimd.collective_compute(
                    "ReduceScatter", mybir.AluOpType.add, replica_groups=G4,
                    ins=[part[cp * SQ:(cp + 1) * SQ, :]],
                    outs=[rsout[cp * 128:(cp + 1) * 128, :]])
                if c > 1:
                    jj = c - 2
                    rt = oup.tile([128, D], BF16, name=f"rt{jj}", tag="rt")
                    nc.gpsimd.dma_start(rt[:], rsout[jj * 128:(jj + 1) * 128, :])
                    rf = oup.tile([128, D], F32, name=f"rf{jj}", tag="rf")
                    nc.gpsimd.tensor_copy(rf[:], rt[:])
                    nc.gpsimd.dma_start(out[jj * 128:(jj + 1) * 128, :], rf[:])
            # o_proj for this 512-row chunk, then ReduceScatter it
            for sm in range(4 * c, 4 * c + 4):
                pos = [opsp.tile([128, SQ], F32, name=f"op{sm}_{i}", tag="op")
                       for i in range(2)]
                for cb in range(2):
                    lhsT = ctx_sb[:, cb * S + sm * 128: cb * S + sm * 128 + 128]
                    for do_ in range(2):
                        nc.tensor.matmul(
                            pos[do_][:],
                            lhsT,
                            wo_sb[:, cb * D + do_ * SQ: cb * D + (do_ + 1) * SQ],
                            start=(cb == 0), stop=(cb == 1))
                ot = obp.tile([128, D], BF16, name=f"ot{sm}", tag="ot")
                nc.vector.tensor_copy(ot[:, 0:SQ], pos[0][:])
                nc.vector.tensor_copy(ot[:, SQ:D], pos[1][:])
                nc.sync.dma_start(part[sm * 128:(sm + 1) * 128, :], ot[:])
            if not dbg and c == NSC - 1:
                nc.gpsimd.collective_compute(
                    "ReduceScatter", mybir.AluOpType.add, replica_groups=G4,
                    ins=[part[c * SQ:(c + 1) * SQ, :]],
                    outs=[rsout[c * 128:(c + 1) * 128, :]])

        if not dbg:
            # ---- output: upcast remaining rsout chunks (2, 3) ----
            with tc.tile_pool(name="oq", bufs=2) as oqp:
                for j in (NSC - 2, NSC - 1):
                    rt = oqp.tile([128, D], BF16, name=f"rt{j}", tag="rt")
                    nc.sync.dma_start(rt[:], rsout[j * 128:(j + 1) * 128, :])
                    rf = oqp.tile([128, D], F32, name=f"rf{j}", tag="rf")
                    nc.vector.tensor_copy(rf[:], rt[:])
                    nc.sync.dma_start(out[j * 128:(j + 1) * 128, :], rf[:])
        else:
            nc.sync.dma_start(t["dbg_xt"][:], xt_sb[:])
            nc.sync.dma_start(t["dbg_q"][:], qrot[0][:])
            nc.sync.dma_start(t["dbg_k"][:], krot[0][:])
            nc.sync.dma_start(t["dbg_v"][:], v_sb[:])
            nc.sync.dma_start(t["dbg_ctx"][:], ctx_sb[:])
            nc.sync.dma_start(t["dbg_wo"][:], wo_sb[:])
    stack.close()


def build_nc(dbg=False):
    nc = bacc.Bacc("TRN2", target_bir_lowering=False, debug=False,
                   enable_asserts=False, num_devices=NCORES)
    t = {}
    if not dbg:
        t["x_sh"] = nc.dram_tensor("x_sh", [SQ, D], F32, kind="ExternalInput").ap()
        t["wq_h"] = nc.dram_tensor("wq_h", [128, D], F32, kind="ExternalInput").ap()
        t["wk_h"] = nc.dram_tensor("wk_h", [128, D], F32, kind="ExternalInput").ap()
        t["wv_h"] = nc.dram_tensor("wv_h", [128, D], F32, kind="ExternalInput").ap()
        t["wo_sh"] = nc.dram_tensor("wo_sh", [128, D], F32,
                                    kind="ExternalInput").ap()
    t["csb"] = nc.dram_tensor("csb", [64, S], BF16, kind="ExternalInput").ap()
    t["snb"] = nc.dram_tensor("snb", [64, S], BF16, kind="ExternalInput").ap()
    t["psw"] = nc.dram_tensor("psw", [128, 128], BF16, kind="ExternalInput").ap()
    t["iden"] = nc.dram_tensor("iden", [128, 128], BF16, kind="ExternalInput").ap()
    t["out"] = nc.dram_tensor("out", [SQ, D], F32, kind="ExternalOutput").ap()

    ikind = "ExternalInput" if dbg else "Internal"
    t["xg"] = nc.dram_tensor("xg", [S, D], BF16, kind=ikind).ap()
    t["qkvg"] = nc.dram_tensor("qkvg", [768, D], BF16, kind=ikind).ap()
    t["woout"] = nc.dram_tensor("woout", [D, CW], BF16, kind=ikind).ap()
    if not dbg:
        t["xcc"] = nc.dram_tensor("xcc", [SQ, D], BF16, kind="Internal").ap()
        t["qkvcc"] = nc.dram_tensor("qkvcc", [384, D], BF16, kind="Internal").ap()
        t["woin"] = nc.dram_tensor("woin", [D, CW], BF16, kind="Internal").ap()
        t["wobh"] = nc.dram_tensor("wobh", [128, D], BF16, kind="Internal").ap()
    t["part"] = nc.dram_tensor("part", [S, D], BF16,
                               kind="ExternalOutput" if dbg else "Internal").ap()
    t["rsout"] = nc.dram_tensor("rsout", [SQ, D], BF16, kind="Internal").ap()
    if dbg:
        t["dbg_xt"] = nc.dram_tensor("dbg_xt", [128, ND * S], BF16,
                                     kind="ExternalOutput").ap()
        t["dbg_q"] = nc.dram_tensor("dbg_q", [128, S], BF16,
                                    kind="ExternalOutput").ap()
        t["dbg_k"] = nc.dram_tensor("dbg_k", [128, S], BF16,
                                    kind="ExternalOutput").ap()
        t["dbg_v"] = nc.dram_tensor("dbg_v", [128, NS * HPC * VW], BF16,
                                    kind="ExternalOutput").ap()
        t["dbg_ctx"] = nc.dram_tensor("dbg_ctx", [128, 2 * S], BF16,
                                      kind="ExternalOutput").ap()
        t["dbg_wo"] = nc.dram_tensor("dbg_wo", [128, 2 * D], BF16,
                                     kind="ExternalOutput").ap()
    with tile.TileContext(nc) as tc:
        _build_kernel(tc, nc, t, dbg=dbg)
    nc.compile()
    return nc


_TABLES = []


def _tables(token_positions):
    if _TABLES:
        return _TABLES[0]
    import ml_dtypes
    pos = np.asarray(token_positions).astype(np.float64)
    inv = ROPE_THETA ** (-2.0 * np.arange(DK // 2, dtype=np.float64) / DK)
    ang = inv[:, None] * pos[None, :]                    # [32, S]
    cs = np.repeat(np.cos(ang), 2, axis=0)               # [64, S]
    sn = np.repeat(np.sin(ang), 2, axis=0)
    sn[0::2] *= -1.0                                     # sign-folded
    csb = cs.astype(ml_dtypes.bfloat16)
    snb = sn.astype(ml_dtypes.bfloat16)
    idx = np.arange(128)
    psw = (idx[:, None] == (idx[None, :] ^ 1)).astype(ml_dtypes.bfloat16)
    iden = np.eye(128, dtype=ml_dtypes.bfloat16)
    _TABLES.append((csb, snb, psw, iden))
    return _TABLES[0]


def make_in_maps(in_features, q_proj_weight, k_proj_weight, v_proj_weight,
                 o_proj_weight, token_positions):
    x = np.asarray(in_features, dtype=np.float32)
    wq = np.asarray(q_proj_weight, dtype=np.float32)
    wk = np.asarray(k_proj_weight, dtype=np.float32)
    wv = np.asarray(v_proj_weight, dtype=np.float32)
    wo = np.asarray(o_proj_weight, dtype=np.float32)
    csb, snb, psw, iden = _tables(token_positions)

    in_maps = []
    for c in range(NCORES):
        b, g = c // 4, c % 4
        hb = c // 4  # pair half index: cores 0-3 upload first halves
        r0 = g * CW + hb * 128
        in_maps.append({
            "x_sh": x[b, g * SQ:(g + 1) * SQ, :],
            "wq_h": wq[r0:r0 + 128, :],
            "wk_h": wk[r0:r0 + 128, :],
            "wv_h": wv[r0:r0 + 128, :],
            "wo_sh": wo[c * 128:(c + 1) * 128, :],
            "csb": csb,
            "snb": snb,
            "psw": psw,
            "iden": iden,
        })
    return in_maps


_NC_CACHE = []
last_exec_ns = None
last_res = None


def kernel(in_features, q_proj_weight, k_proj_weight, v_proj_weight,
           o_proj_weight, token_positions, d_model=1024, num_heads=16,
           **_ignored):
    global last_exec_ns, last_res
    assert int(d_model) == D and int(num_heads) == H
    in_maps = make_in_maps(in_features, q_proj_weight, k_proj_weight,
                           v_proj_weight, o_proj_weight, token_positions)
    if not _NC_CACHE:
        _NC_CACHE.append(build_nc())
    nc = _NC_CACHE[0]
    trace = bool(int(os.environ.get("KERNEL_TRACE", "0")))
    res = bass_utils.run_bass_kernel_spmd(nc, in_maps,
                                          core_ids=list(range(NCORES)),
                                          trace=trace)
    last_exec_ns = res.exec_time_ns
    last_res = res
    out = np.empty((B, S, D), np.float32)
    for c in range(NCORES):
        b, k = c // 4, c % 4
        p = res.results[c]["out"].astype(np.float32)
        for j in range(NSC):
            out[b, j * SQ + k * 128: j * SQ + (k + 1) * 128] = \
                p[j * 128:(j + 1) * 128]
    return out
